# revision 15
# baseline (speedup 1.0000x reference)
"""MoE kernel for Trainium2 (8 NeuronCores) — 8-way feature-split.

Strategy (feature-parallel over DFF, skew-immune):
  - Host sorts the T=4096 tokens by dispatch_order into per-expert column
    blocks (padded to a multiple of 8).  EVERY core sees all token columns,
    but core c computes only its 4 of the 32 DFF f-chunks (512 of 4096 ff
    dims) for every expert:  h_f = gelu(x @ W1[:, f] + b1[f]) entirely
    on-core, partial y_c = sum_f h_f @ W2[f, :].  The host sums the 8
    partial outputs and adds b2.  This is an exact decomposition; per-core
    compute is sum_e ceil8(count_e) columns (~517-equivalent) instead of
    8*max_e(count_e) (~576), and weight traffic stays 16 MB/core (each
    core reads 1/8th of every expert's W1/W2).
  - Device loop per expert block: phase 1 k-outer (PSUM holds the 4
    f-chunks while the 8 k-tiles stream in -> the PE starts ~1us into the
    kernel, paced by DMA only for block 0), then phase 2 chunk-outer
    (yT partial = W2-slice @ h), partial y DMA'd out as float16.
  - All DMAs are contiguous [128, N] slabs (host packs x block-major
    k-inner, y block-major dm-inner, w1/w2 per-block slabs).

Self-contained: hardcodes all shapes from the problem spec.
"""

import os
import sys
from contextlib import ExitStack

import numpy as np

for _p in ("/opt/trn_rl_repo",):
    if _p not in sys.path:
        sys.path.insert(0, _p)

import concourse.bass as bass  # noqa: E402
import concourse.tile as tile  # noqa: E402
from concourse import mybir  # noqa: E402
from concourse.bass_utils import run_bass_kernel_spmd  # noqa: E402

# ---------------------------------------------------------------------------
# Workaround for this walrus build: a Drain instruction with >1 sem wait
# fails codegen ("Too many sync wait commands").  Replace the Tile
# kernel-tail drain with single-wait SP nops followed by a bare drain.
# ---------------------------------------------------------------------------


def _patched_drain_and_barrier(self, tick_clock, wait_clock):
    from concourse.vector_clock import ScopedClock

    nc = self.nc
    probe = nc.sync.nop(nofuse=True)
    wait_clock.add_sem_waits(probe.ins, ScopedClock({None: tick_clock.global_clock}))
    si = probe.ins.sync_info
    waits = list(si.on_wait) if si and si.on_wait else []
    probe.ins.sync_info = mybir.SyncInfo(on_wait=waits[:1], on_update=[])
    for w in waits[1:]:
        n = nc.sync.nop(nofuse=True)
        n.ins.sync_info = mybir.SyncInfo(on_wait=[w], on_update=[])

    nc.sync.drain()
    nc.all_engine_barrier()
    assert self.sems is not None
    popped = nc._tile_sem_poison_stack.pop()
    assert popped is self._sem_poison
    nc.clear_and_free_semaphores(list(self.sems.allocated().values()))
    nc.all_engine_barrier()


tile.TileContext._drain_and_barrier = _patched_drain_and_barrier


def _split_excess_sync_waits(nc, max_waits=1):
    """This walrus build only encodes one sem wait per instruction.  Hoist
    excess waits onto same-engine nops inserted immediately before."""
    for f in nc.m.functions:
        for bb in f.blocks:
            out = []
            for inst in bb.instructions:
                si = inst.sync_info
                if si and si.on_wait and len(si.on_wait) > max_waits:
                    waits = list(si.on_wait)
                    for i in range(max_waits, len(waits), max_waits):
                        n = mybir.InstNoOp(
                            name=f"{inst.name}-waitsplit-{i}", ins=[], outs=[]
                        )
                        n.engine = inst.engine
                        n.sync_info = mybir.SyncInfo(
                            on_wait=waits[i : i + max_waits], on_update=[]
                        )
                        out.append(n)
                    inst.sync_info = mybir.SyncInfo(
                        on_wait=waits[:max_waits], on_update=list(si.on_update or [])
                    )
                out.append(inst)
            bb.instructions[:] = out


# ---------------------------------------------------------------------------

NUM_EXPERTS = 8
D = 1024
DFF = 4096
N_CORES = 8
KD = D // 128  # 8 contraction chunks for matmul 1
DM = D // 128  # 8 output chunks for matmul 2
FPC = (DFF // 128) // N_CORES  # 4 f-chunks per core
WCOL = KD * FPC * 128  # 4096 packed weight columns per block (w1 and w2)

F32 = mybir.dt.float32
F16 = mybir.dt.float16

LAST_EXEC_NS = None
LAST_RESULT = None

_NC_CACHE = {}


def _chunks(S):
    """Split S columns into <=512-wide chunks (PSUM bank limit), sizes
    multiple of 8, all >=256 when S allows (hides LDWEIGHTS)."""
    n = max(1, -(-S // 512))
    base = -(-(-(-S // n)) // 8) * 8
    out = []
    c0 = 0
    while c0 < S:
        cn = min(base, S - c0)
        out.append((c0, cn))
        c0 += cn
    return out


def _build_nc(sizes):
    nb = len(sizes)
    C = sum(sizes)
    nc = bass.Bass()
    xk = nc.declare_dram_parameter("xk", [128, KD * C], F16, isOutput=False)
    w1 = nc.declare_dram_parameter("w1", [128, nb * WCOL], F16, isOutput=False)
    w2 = nc.declare_dram_parameter("w2", [128, nb * WCOL], F16, isOutput=False)
    b1 = nc.declare_dram_parameter("b1", [128, nb * FPC], F32, isOutput=False)
    yk = nc.declare_dram_parameter("yk", [128, DM * C], F16, isOutput=True)

    gelu = mybir.ActivationFunctionType.Gelu_apprx_tanh
    xoff = [0]
    for S in sizes:
        xoff.append(xoff[-1] + KD * S)
    yoff = [0]
    for S in sizes:
        yoff.append(yoff[-1] + DM * S)

    with ExitStack() as ctx:
        tc = ctx.enter_context(tile.TileContext(nc))
        # Pool depths sized for ~512-col blocks; scale down for extremely
        # skewed dispatch so the pools always fit in SBUF (~190KB/partition).
        big = max(sizes) > 1024
        cpool = ctx.enter_context(tc.tile_pool(name="const", bufs=1))
        wpool = ctx.enter_context(tc.tile_pool(name="w", bufs=1 if big else 3))
        xpool = ctx.enter_context(tc.tile_pool(name="x", bufs=1 if big else 3))
        ypool = ctx.enter_context(tc.tile_pool(name="y", bufs=1 if big else 2))
        hpool = ctx.enter_context(tc.tile_pool(name="h", bufs=FPC if big else 2 * FPC))
        pspool = ctx.enter_context(tc.tile_pool(name="ps", bufs=4, space="PSUM"))

        xts, w1ts, w2ts = {}, {}, {}

        def prefetch(e):
            S = sizes[e]
            xt = xpool.tile([128, KD * S], F16, name="xt", tag="x")
            w1t = wpool.tile([128, WCOL], F16, name="w1t", tag="w1")
            if e == 0:
                # Block 0 feeds the PE while it streams in: per-k pieces,
                # x[k] and w1[k] alternated across BOTH HWDGE queues in
                # strict k order, so each queue's FIFO delivers the k-steps
                # fairly and no prefetch traffic can overtake them.
                qs = [nc.scalar, nc.sync]
                for k in range(KD):
                    qs[k % 2].dma_start(
                        xt[:, k * S : (k + 1) * S], xk[:, k * S : (k + 1) * S]
                    )
                    qs[(k + 1) % 2].dma_start(
                        w1t[:, k * 512 : (k + 1) * 512],
                        w1[:, k * 512 : (k + 1) * 512],
                    )
            else:
                nc.scalar.dma_start(xt[:], xk[:, xoff[e] : xoff[e + 1]])
                nc.sync.dma_start(w1t[:], w1[:, e * WCOL : (e + 1) * WCOL])
            xts[e] = xt
            w1ts[e] = w1t
            w2t = wpool.tile([128, WCOL], F16, name="w2t", tag="w2")
            if e == 0:
                # Four slices fill the sync DGE ring (transfers in the ring
                # progress round-robin, so 4 slices get 4 shares of HBM
                # bandwidth AND block later prefetch from entering the ring
                # until block 0's w2 has landed).
                for q in range(4):
                    nc.sync.dma_start(
                        w2t[:, q * 1024 : (q + 1) * 1024],
                        w2[:, q * 1024 : (q + 1) * 1024],
                    )
            else:
                nc.sync.dma_start(w2t[:], w2[:, e * WCOL : (e + 1) * WCOL])
            w2ts[e] = w2t

        # b1 first on sync: 16KB, lands ~1us in, so the first activation
        # never waits on it behind megabytes of weight prefetch.
        b1_sb = cpool.tile([128, nb * FPC], F32, name="b1_sb")
        nc.sync.dma_start(b1_sb[:], b1[:, :])
        prefetch(0)
        # Dummy gelu on a preamble-memset const tile: pulls the scalar
        # engine's ACT_TABLE_LOAD (~1.5us) off the phase-1 -> phase-2
        # critical path without waiting on any DMA.
        warm_out = cpool.tile([128, 1], F16, name="warm_out")
        nc.scalar.activation(
            warm_out[:],
            nc.const_aps.tensor(0.0, (128, 1), F32),
            gelu,
            bias=0.0,
            scale=1.0,
        )
        if nb > 1:
            prefetch(1)

        for e, S in enumerate(sizes):
            if e >= 1 and e + 2 < nb:
                prefetch(e + 2)
            xt, w1t, w2t = xts.pop(e), w1ts.pop(e), w2ts.pop(e)
            chunks = _chunks(S)

            # ---- phase 1: h_f = gelu(x @ W1[:,f] + b1[f]), k-outer ----
            # Chunks processed in groups of 2 using both PSUM tag rings (8
            # banks): doubles the PE work per arriving k-tile, so block 0
            # stays PE-bound even while its x/w1 stream in.
            hs = [hpool.tile([128, S], F16, name="h", tag="h") for _ in range(FPC)]
            for g0 in range(0, len(chunks), 2):
                grp = chunks[g0 : g0 + 2]
                pss = [
                    [
                        pspool.tile(
                            [128, cn], F32, name="ps1", tag=("p1", "p2")[gi]
                        )
                        for f in range(FPC)
                    ]
                    for gi, (c0, cn) in enumerate(grp)
                ]
                for k in range(KD):
                    for f in range(FPC):
                        for gi, (c0, cn) in enumerate(grp):
                            nc.tensor.matmul(
                                pss[gi][f][:, :],
                                w1t[:, k * 512 + f * 128 : k * 512 + (f + 1) * 128],
                                xt[:, k * S + c0 : k * S + c0 + cn],
                                start=(k == 0),
                                stop=(k == KD - 1),
                            )
                for gi, (c0, cn) in enumerate(grp):
                    for f in range(FPC):
                        nc.scalar.activation(
                            hs[f][:, c0 : c0 + cn],
                            pss[gi][f][:, :],
                            gelu,
                            bias=b1_sb[:, e * FPC + f : e * FPC + f + 1],
                            scale=1.0,
                        )

            # ---- phase 2: y_partial = sum_f h_f @ W2[f,:], chunk-outer ----
            if e == 0 and nb > 2:
                # prefetch(2) deferred to here so its transfers cannot share
                # the DGE rings with block 0's critical w2 load.
                prefetch(2)
            yt = ypool.tile([128, DM * S], F16, name="yt", tag="y")
            last_ci = len(chunks) - 1
            for ci, (c0, cn) in enumerate(chunks):
                for dm in range(DM):
                    ps2 = pspool.tile([128, cn], F32, name="ps2", tag="p2")
                    for f in range(FPC):
                        nc.tensor.matmul(
                            ps2[:, :],
                            w2t[:, f * 1024 + dm * 128 : f * 1024 + (dm + 1) * 128],
                            hs[f][:, c0 : c0 + cn],
                            start=(f == 0),
                            stop=(f == FPC - 1),
                        )
                    nc.vector.tensor_scalar_add(
                        yt[:, dm * S + c0 : dm * S + c0 + cn], ps2[:, :], 0.0
                    )
                    if ci == last_ci:
                        # y out on the scalar queue: issued after this block's
                        # activations, waits only on copies already in flight.
                        # The final block's last two dm go out singly so the
                        # kernel-tail transfer is as small as possible.
                        tail = e == nb - 1 and dm >= 6
                        if tail:
                            nc.scalar.dma_start(
                                yk[:, yoff[e] + dm * S : yoff[e] + (dm + 1) * S],
                                yt[:, dm * S : (dm + 1) * S],
                            )
                        elif dm % 2 == 1 and not (e == nb - 1 and dm == 7):
                            nc.scalar.dma_start(
                                yk[:, yoff[e] + (dm - 1) * S : yoff[e] + (dm + 1) * S],
                                yt[:, (dm - 1) * S : (dm + 1) * S],
                            )

    _split_excess_sync_waits(nc)
    return nc


def _enable_trace_hooks():
    """Register the NTFF profile hook (missing antenv.axon_hooks shim)."""
    import types

    if "antenv.axon_hooks" not in sys.modules:
        mod = types.ModuleType("antenv.axon_hooks")
        mod._hook = None

        def set_axon_ntff_profile_hook(h):
            mod._hook = h

        def get_axon_ntff_profile_hook():
            return mod._hook

        mod.set_axon_ntff_profile_hook = set_axon_ntff_profile_hook
        mod.get_axon_ntff_profile_hook = get_axon_ntff_profile_hook
        sys.modules["antenv.axon_hooks"] = mod
        import antenv

        antenv.axon_hooks = mod
    import antenv.axon_hooks as ah

    if ah.get_axon_ntff_profile_hook() is None:
        from trn_agent_boot.trn_boot import _ntff_profile_via_ctypes

        ah.set_axon_ntff_profile_hook(
            _ntff_profile_via_ctypes("/opt/axon/libaxon_pjrt.so")
        )
    import concourse.bass_utils as bu

    bu.upload_artifacts = lambda tmpdir: "local://skipped"


def kernel(inputs, w1, b1, w2, b2, dispatch_order):
    global LAST_EXEC_NS, LAST_RESULT

    inputs = np.asarray(inputs, dtype=np.float32)
    w1 = np.asarray(w1, dtype=np.float32)
    b1 = np.asarray(b1, dtype=np.float32)
    w2 = np.asarray(w2, dtype=np.float32)
    b2 = np.asarray(b2, dtype=np.float32)
    disp = np.asarray(dispatch_order).astype(np.int64)

    B, Sq, _ = inputs.shape
    T = B * Sq
    x = inputs.reshape(T, D)

    order = np.argsort(disp, kind="stable")
    counts = np.bincount(disp, minlength=NUM_EXPERTS)
    starts = np.zeros(NUM_EXPERTS + 1, dtype=np.int64)
    np.cumsum(counts, out=starts[1:])

    # blocks: experts with tokens, processed big->small (tail = smallest)
    blocks = sorted(
        (e for e in range(NUM_EXPERTS) if counts[e] > 0),
        key=lambda e: (-counts[e], e),
    )
    sizes = tuple(int(-(-counts[e] // 8) * 8) for e in blocks)
    offs = np.zeros(len(sizes) + 1, dtype=np.int64)
    np.cumsum(sizes, out=offs[1:])
    C = int(offs[-1])

    key = sizes
    if key not in _NC_CACHE:
        _NC_CACHE[key] = _build_nc(sizes)
    nc = _NC_CACHE[key]

    # ---- pack x: per block, [128, KD*S] k-inner slabs, concatenated ----
    xk_arr = np.zeros((128, KD * C), dtype=np.float16)
    tok_lists = []
    for bi, e in enumerate(blocks):
        toks = order[starts[e] : starts[e + 1]]
        tok_lists.append(toks)
        S = sizes[bi]
        xb = np.zeros((128, KD, S), dtype=np.float16)
        # x[toks] is [n, 1024]; feature dim k*128+p -> [k, p, n] -> [p, k, n]
        xb[:, :, : len(toks)] = (
            x[toks].T.reshape(KD, 128, len(toks)).transpose(1, 0, 2)
        )
        xk_arr[:, KD * offs[bi] : KD * offs[bi + 1]] = xb.reshape(128, KD * S)

    # ---- per-core weight packs: core c owns f-chunks [c*FPC, (c+1)*FPC) ----
    nb = len(blocks)
    w1_blocks = w1[blocks]  # [nb, 1024, 4096]
    w2_blocks = w2[blocks]  # [nb, 4096, 1024]
    b1_blocks = b1[blocks]  # [nb, 4096]
    in_maps = []
    for c in range(N_CORES):
        ff = slice(c * FPC * 128, (c + 1) * FPC * 128)
        # w1p[p, e*WCOL + k*512 + fl*128 + j] = w1[e][k*128+p, ff.start+fl*128+j]
        w1p = np.ascontiguousarray(
            w1_blocks[:, :, ff]
            .reshape(nb, KD, 128, FPC, 128)
            .transpose(2, 0, 1, 3, 4)
            .reshape(128, nb * WCOL)
        ).astype(np.float16)
        # w2p[p, e*WCOL + fl*1024 + dm*128 + j] = w2[e][ff.start+fl*128+p, dm*128+j]
        w2p = np.ascontiguousarray(
            w2_blocks[:, ff, :]
            .reshape(nb, FPC, 128, DM, 128)
            .transpose(2, 0, 1, 3, 4)
            .reshape(128, nb * WCOL)
        ).astype(np.float16)
        # b1p[p, e*FPC + fl] = b1[e][ff.start + fl*128 + p]
        b1p = np.ascontiguousarray(
            b1_blocks[:, ff].reshape(nb, FPC, 128).transpose(2, 0, 1).reshape(
                128, nb * FPC
            )
        ).astype(np.float32)
        in_maps.append({"xk": xk_arr, "w1": w1p, "w2": w2p, "b1": b1p})

    trace = os.environ.get("MOE_TRACE") == "1"
    kwargs = {}
    if trace:
        _enable_trace_hooks()
        kwargs["trace"] = True
        tmpdir = os.environ.get("MOE_TRACE_DIR")
        if tmpdir:
            os.makedirs(tmpdir, exist_ok=True)
            kwargs["tmpdir"] = tmpdir

    res = run_bass_kernel_spmd(nc, in_maps, list(range(N_CORES)), **kwargs)
    LAST_RESULT = res
    LAST_EXEC_NS = res.exec_time_ns

    # ---- gather: sum the 8 partial outputs, add b2, unsort ----
    ysum = np.zeros((128, DM * C), dtype=np.float32)
    for c in range(N_CORES):
        ysum += res.results[c]["yk"].astype(np.float32)

    out = np.empty((T, D), dtype=np.float32)
    for bi, e in enumerate(blocks):
        toks = tok_lists[bi]
        S = sizes[bi]
        yb = (
            ysum[:, DM * offs[bi] : DM * offs[bi + 1]]
            .reshape(128, DM, S)
            .transpose(1, 0, 2)
            .reshape(D, S)
        )
        out[toks] = yb[:, : len(toks)].T + b2[e][None, :]
    return out.reshape(B, Sq, D)


# revision 19
# speedup vs baseline: 1.0147x; 1.0147x over previous
"""MoE kernel for Trainium2 (8 NeuronCores) — 8-way feature-split.

Strategy (feature-parallel over DFF, skew-immune):
  - Host sorts the T=4096 tokens by dispatch_order into per-expert column
    blocks (padded to a multiple of 8).  EVERY core sees all token columns,
    but core c computes only its 4 of the 32 DFF f-chunks (512 of 4096 ff
    dims) for every expert:  h_f = gelu(x @ W1[:, f] + b1[f]) entirely
    on-core, partial y_c = sum_f h_f @ W2[f, :].  The host sums the 8
    partial outputs and adds b2.  This is an exact decomposition; per-core
    compute is sum_e ceil8(count_e) columns (~517-equivalent) instead of
    8*max_e(count_e) (~576), and weight traffic stays 16 MB/core (each
    core reads 1/8th of every expert's W1/W2).
  - Device loop per expert block: phase 1 k-outer (PSUM holds the 4
    f-chunks while the 8 k-tiles stream in -> the PE starts ~1us into the
    kernel, paced by DMA only for block 0), then phase 2 chunk-outer
    (yT partial = W2-slice @ h), partial y DMA'd out as float16.
  - All DMAs are contiguous [128, N] slabs (host packs x block-major
    k-inner, y block-major dm-inner, w1/w2 per-block slabs).

Self-contained: hardcodes all shapes from the problem spec.
"""

import os
import sys
from contextlib import ExitStack

import numpy as np

for _p in ("/opt/trn_rl_repo",):
    if _p not in sys.path:
        sys.path.insert(0, _p)

import concourse.bass as bass  # noqa: E402
import concourse.tile as tile  # noqa: E402
from concourse import mybir  # noqa: E402
from concourse.bass_utils import run_bass_kernel_spmd  # noqa: E402

# ---------------------------------------------------------------------------
# Workaround for this walrus build: a Drain instruction with >1 sem wait
# fails codegen ("Too many sync wait commands").  Replace the Tile
# kernel-tail drain with single-wait SP nops followed by a bare drain.
# ---------------------------------------------------------------------------


def _patched_drain_and_barrier(self, tick_clock, wait_clock):
    from concourse.vector_clock import ScopedClock

    nc = self.nc
    probe = nc.sync.nop(nofuse=True)
    wait_clock.add_sem_waits(probe.ins, ScopedClock({None: tick_clock.global_clock}))
    si = probe.ins.sync_info
    waits = list(si.on_wait) if si and si.on_wait else []
    probe.ins.sync_info = mybir.SyncInfo(on_wait=waits[:1], on_update=[])
    for w in waits[1:]:
        n = nc.sync.nop(nofuse=True)
        n.ins.sync_info = mybir.SyncInfo(on_wait=[w], on_update=[])

    nc.sync.drain()
    nc.all_engine_barrier()
    assert self.sems is not None
    popped = nc._tile_sem_poison_stack.pop()
    assert popped is self._sem_poison
    nc.clear_and_free_semaphores(list(self.sems.allocated().values()))
    nc.all_engine_barrier()


tile.TileContext._drain_and_barrier = _patched_drain_and_barrier


def _split_excess_sync_waits(nc, max_waits=1):
    """This walrus build only encodes one sem wait per instruction.  Hoist
    excess waits onto same-engine nops inserted immediately before."""
    for f in nc.m.functions:
        for bb in f.blocks:
            out = []
            for inst in bb.instructions:
                si = inst.sync_info
                if si and si.on_wait and len(si.on_wait) > max_waits:
                    waits = list(si.on_wait)
                    for i in range(max_waits, len(waits), max_waits):
                        n = mybir.InstNoOp(
                            name=f"{inst.name}-waitsplit-{i}", ins=[], outs=[]
                        )
                        n.engine = inst.engine
                        n.sync_info = mybir.SyncInfo(
                            on_wait=waits[i : i + max_waits], on_update=[]
                        )
                        out.append(n)
                    inst.sync_info = mybir.SyncInfo(
                        on_wait=waits[:max_waits], on_update=list(si.on_update or [])
                    )
                out.append(inst)
            bb.instructions[:] = out


# ---------------------------------------------------------------------------

NUM_EXPERTS = 8
D = 1024
DFF = 4096
N_CORES = 8
KD = D // 128  # 8 contraction chunks for matmul 1
DM = D // 128  # 8 output chunks for matmul 2
FPC = (DFF // 128) // N_CORES  # 4 f-chunks per core
WCOL = KD * FPC * 128  # 4096 packed weight columns per block (w1 and w2)

F32 = mybir.dt.float32
F16 = mybir.dt.float16

LAST_EXEC_NS = None
LAST_RESULT = None

_NC_CACHE = {}


def _chunks(S):
    """Split S columns into <=512-wide chunks (PSUM bank limit), sizes
    multiple of 8, all >=256 when S allows (hides LDWEIGHTS)."""
    n = max(1, -(-S // 512))
    base = -(-(-(-S // n)) // 8) * 8
    out = []
    c0 = 0
    while c0 < S:
        cn = min(base, S - c0)
        out.append((c0, cn))
        c0 += cn
    return out


def _build_nc(sizes):
    nb = len(sizes)
    C = sum(sizes)
    nc = bass.Bass()
    xk = nc.declare_dram_parameter("xk", [128, KD * C], F16, isOutput=False)
    w1 = nc.declare_dram_parameter("w1", [128, nb * WCOL], F16, isOutput=False)
    w2 = nc.declare_dram_parameter("w2", [128, nb * WCOL], F16, isOutput=False)
    b1 = nc.declare_dram_parameter("b1", [128, nb * FPC], F32, isOutput=False)
    yk = nc.declare_dram_parameter("yk", [128, DM * C], F16, isOutput=True)

    gelu = mybir.ActivationFunctionType.Gelu_apprx_tanh
    xoff = [0]
    for S in sizes:
        xoff.append(xoff[-1] + KD * S)
    yoff = [0]
    for S in sizes:
        yoff.append(yoff[-1] + DM * S)

    with ExitStack() as ctx:
        tc = ctx.enter_context(tile.TileContext(nc))
        # Pool depths sized for ~512-col blocks; scale down for extremely
        # skewed dispatch so the pools always fit in SBUF (~190KB/partition).
        big = max(sizes) > 1024
        cpool = ctx.enter_context(tc.tile_pool(name="const", bufs=1))
        wpool = ctx.enter_context(tc.tile_pool(name="w", bufs=1 if big else 2))
        xpool = ctx.enter_context(tc.tile_pool(name="x", bufs=1 if big else 3))
        ypool = ctx.enter_context(tc.tile_pool(name="y", bufs=1 if big else 2))
        hpool = ctx.enter_context(tc.tile_pool(name="h", bufs=FPC if big else 2 * FPC))
        pspool = ctx.enter_context(tc.tile_pool(name="ps", bufs=4, space="PSUM"))

        xts, w1ts, w2ts = {}, {}, {}
        # Startup k-batches: 4 triggers (queue holds ~4 outstanding) sized so
        # early k-tiles land fast while the PE is still cold-clocked.
        KB = [(0, 1), (1, 1), (2, 2), (4, KD - 4)] if KD == 8 else [(0, KD)]

        def prefetch(e):
            S = sizes[e]
            # x: scalar-engine HWDGE queue (shared with activations; the
            # ring depth of 3 keeps the trigger's sem wait long-satisfied).
            xt = xpool.tile([128, KD * S], F16, name="xt", tag="x")
            if e == 0:
                # split per k-batch so the PE can start on k=0 at ~1us
                for k0, kn in KB:
                    nc.scalar.dma_start(
                        xt[:, k0 * S : (k0 + kn) * S], xk[:, k0 * S : (k0 + kn) * S]
                    )
            else:
                nc.scalar.dma_start(xt[:], xk[:, xoff[e] : xoff[e + 1]])
            xts[e] = xt
            w1t = wpool.tile([128, WCOL], F16, name="w1t", tag="w1")
            if e == 0:
                for k0, kn in KB:
                    nc.sync.dma_start(
                        w1t[:, k0 * 512 : (k0 + kn) * 512],
                        w1[:, k0 * 512 : (k0 + kn) * 512],
                    )
            else:
                nc.sync.dma_start(w1t[:], w1[:, e * WCOL : (e + 1) * WCOL])
            w1ts[e] = w1t
            w2t = wpool.tile([128, WCOL], F16, name="w2t", tag="w2")
            nc.sync.dma_start(w2t[:], w2[:, e * WCOL : (e + 1) * WCOL])
            w2ts[e] = w2t

        prefetch(0)
        b1_sb = cpool.tile([128, nb * FPC], F32, name="b1_sb")
        nc.scalar.dma_start(b1_sb[:], b1[:, :])
        if nb > 1:
            prefetch(1)

        for e, S in enumerate(sizes):
            if e + 2 < nb:
                prefetch(e + 2)
            xt, w1t, w2t = xts.pop(e), w1ts.pop(e), w2ts.pop(e)
            chunks = _chunks(S)

            # ---- phase 1: h_f = gelu(x @ W1[:,f] + b1[f]), k-outer ----
            # Chunks processed in groups of 2 using both PSUM tag rings (8
            # banks): doubles the PE work per arriving k-tile, so block 0
            # stays PE-bound even while its x/w1 stream in.
            hs = [hpool.tile([128, S], F16, name="h", tag="h") for _ in range(FPC)]
            for g0 in range(0, len(chunks), 2):
                grp = chunks[g0 : g0 + 2]
                pss = [
                    [
                        pspool.tile(
                            [128, cn], F32, name="ps1", tag=("p1", "p2")[gi]
                        )
                        for f in range(FPC)
                    ]
                    for gi, (c0, cn) in enumerate(grp)
                ]
                for k in range(KD):
                    for f in range(FPC):
                        for gi, (c0, cn) in enumerate(grp):
                            nc.tensor.matmul(
                                pss[gi][f][:, :],
                                w1t[:, k * 512 + f * 128 : k * 512 + (f + 1) * 128],
                                xt[:, k * S + c0 : k * S + c0 + cn],
                                start=(k == 0),
                                stop=(k == KD - 1),
                            )
                for gi, (c0, cn) in enumerate(grp):
                    for f in range(FPC):
                        nc.scalar.activation(
                            hs[f][:, c0 : c0 + cn],
                            pss[gi][f][:, :],
                            gelu,
                            bias=b1_sb[:, e * FPC + f : e * FPC + f + 1],
                            scale=1.0,
                        )

            # ---- phase 2: y_partial = sum_f h_f @ W2[f,:], chunk-outer ----
            yt = ypool.tile([128, DM * S], F16, name="yt", tag="y")
            last_ci = len(chunks) - 1
            for ci, (c0, cn) in enumerate(chunks):
                for dm in range(DM):
                    ps2 = pspool.tile([128, cn], F32, name="ps2", tag="p2")
                    for f in range(FPC):
                        nc.tensor.matmul(
                            ps2[:, :],
                            w2t[:, f * 1024 + dm * 128 : f * 1024 + (dm + 1) * 128],
                            hs[f][:, c0 : c0 + cn],
                            start=(f == 0),
                            stop=(f == FPC - 1),
                        )
                    nc.vector.tensor_scalar_add(
                        yt[:, dm * S + c0 : dm * S + c0 + cn], ps2[:, :], 0.0
                    )
                    if ci == last_ci:
                        # y out on the scalar queue: issued after this block's
                        # activations, waits only on copies already in flight.
                        # The final block's last two dm go out singly so the
                        # kernel-tail transfer is as small as possible.
                        tail = e == nb - 1 and dm >= 6
                        if tail:
                            nc.scalar.dma_start(
                                yk[:, yoff[e] + dm * S : yoff[e] + (dm + 1) * S],
                                yt[:, dm * S : (dm + 1) * S],
                            )
                        elif dm % 2 == 1 and not (e == nb - 1 and dm == 7):
                            nc.scalar.dma_start(
                                yk[:, yoff[e] + (dm - 1) * S : yoff[e] + (dm + 1) * S],
                                yt[:, (dm - 1) * S : (dm + 1) * S],
                            )

    _split_excess_sync_waits(nc)
    return nc


def _enable_trace_hooks():
    """Register the NTFF profile hook (missing antenv.axon_hooks shim)."""
    import types

    if "antenv.axon_hooks" not in sys.modules:
        mod = types.ModuleType("antenv.axon_hooks")
        mod._hook = None

        def set_axon_ntff_profile_hook(h):
            mod._hook = h

        def get_axon_ntff_profile_hook():
            return mod._hook

        mod.set_axon_ntff_profile_hook = set_axon_ntff_profile_hook
        mod.get_axon_ntff_profile_hook = get_axon_ntff_profile_hook
        sys.modules["antenv.axon_hooks"] = mod
        import antenv

        antenv.axon_hooks = mod
    import antenv.axon_hooks as ah

    if ah.get_axon_ntff_profile_hook() is None:
        from trn_agent_boot.trn_boot import _ntff_profile_via_ctypes

        ah.set_axon_ntff_profile_hook(
            _ntff_profile_via_ctypes("/opt/axon/libaxon_pjrt.so")
        )
    import concourse.bass_utils as bu

    bu.upload_artifacts = lambda tmpdir: "local://skipped"


def kernel(inputs, w1, b1, w2, b2, dispatch_order):
    global LAST_EXEC_NS, LAST_RESULT

    inputs = np.asarray(inputs, dtype=np.float32)
    w1 = np.asarray(w1, dtype=np.float32)
    b1 = np.asarray(b1, dtype=np.float32)
    w2 = np.asarray(w2, dtype=np.float32)
    b2 = np.asarray(b2, dtype=np.float32)
    disp = np.asarray(dispatch_order).astype(np.int64)

    B, Sq, _ = inputs.shape
    T = B * Sq
    x = inputs.reshape(T, D)

    order = np.argsort(disp, kind="stable")
    counts = np.bincount(disp, minlength=NUM_EXPERTS)
    starts = np.zeros(NUM_EXPERTS + 1, dtype=np.int64)
    np.cumsum(counts, out=starts[1:])

    # blocks: experts with tokens, processed big->small (tail = smallest)
    blocks = sorted(
        (e for e in range(NUM_EXPERTS) if counts[e] > 0),
        key=lambda e: (-counts[e], e),
    )
    sizes = tuple(int(-(-counts[e] // 8) * 8) for e in blocks)
    offs = np.zeros(len(sizes) + 1, dtype=np.int64)
    np.cumsum(sizes, out=offs[1:])
    C = int(offs[-1])

    key = sizes
    if key not in _NC_CACHE:
        _NC_CACHE[key] = _build_nc(sizes)
    nc = _NC_CACHE[key]

    # ---- pack x: per block, [128, KD*S] k-inner slabs, concatenated ----
    xk_arr = np.zeros((128, KD * C), dtype=np.float16)
    tok_lists = []
    for bi, e in enumerate(blocks):
        toks = order[starts[e] : starts[e + 1]]
        tok_lists.append(toks)
        S = sizes[bi]
        xb = np.zeros((128, KD, S), dtype=np.float16)
        # x[toks] is [n, 1024]; feature dim k*128+p -> [k, p, n] -> [p, k, n]
        xb[:, :, : len(toks)] = (
            x[toks].T.reshape(KD, 128, len(toks)).transpose(1, 0, 2)
        )
        xk_arr[:, KD * offs[bi] : KD * offs[bi + 1]] = xb.reshape(128, KD * S)

    # ---- per-core weight packs: core c owns f-chunks [c*FPC, (c+1)*FPC) ----
    nb = len(blocks)
    w1_blocks = w1[blocks]  # [nb, 1024, 4096]
    w2_blocks = w2[blocks]  # [nb, 4096, 1024]
    b1_blocks = b1[blocks]  # [nb, 4096]
    in_maps = []
    for c in range(N_CORES):
        ff = slice(c * FPC * 128, (c + 1) * FPC * 128)
        # w1p[p, e*WCOL + k*512 + fl*128 + j] = w1[e][k*128+p, ff.start+fl*128+j]
        w1p = np.ascontiguousarray(
            w1_blocks[:, :, ff]
            .reshape(nb, KD, 128, FPC, 128)
            .transpose(2, 0, 1, 3, 4)
            .reshape(128, nb * WCOL)
        ).astype(np.float16)
        # w2p[p, e*WCOL + fl*1024 + dm*128 + j] = w2[e][ff.start+fl*128+p, dm*128+j]
        w2p = np.ascontiguousarray(
            w2_blocks[:, ff, :]
            .reshape(nb, FPC, 128, DM, 128)
            .transpose(2, 0, 1, 3, 4)
            .reshape(128, nb * WCOL)
        ).astype(np.float16)
        # b1p[p, e*FPC + fl] = b1[e][ff.start + fl*128 + p]
        b1p = np.ascontiguousarray(
            b1_blocks[:, ff].reshape(nb, FPC, 128).transpose(2, 0, 1).reshape(
                128, nb * FPC
            )
        ).astype(np.float32)
        in_maps.append({"xk": xk_arr, "w1": w1p, "w2": w2p, "b1": b1p})

    trace = os.environ.get("MOE_TRACE") == "1"
    kwargs = {}
    if trace:
        _enable_trace_hooks()
        kwargs["trace"] = True
        tmpdir = os.environ.get("MOE_TRACE_DIR")
        if tmpdir:
            os.makedirs(tmpdir, exist_ok=True)
            kwargs["tmpdir"] = tmpdir

    res = run_bass_kernel_spmd(nc, in_maps, list(range(N_CORES)), **kwargs)
    LAST_RESULT = res
    LAST_EXEC_NS = res.exec_time_ns

    # ---- gather: sum the 8 partial outputs, add b2, unsort ----
    ysum = np.zeros((128, DM * C), dtype=np.float32)
    for c in range(N_CORES):
        ysum += res.results[c]["yk"].astype(np.float32)

    out = np.empty((T, D), dtype=np.float32)
    for bi, e in enumerate(blocks):
        toks = tok_lists[bi]
        S = sizes[bi]
        yb = (
            ysum[:, DM * offs[bi] : DM * offs[bi + 1]]
            .reshape(128, DM, S)
            .transpose(1, 0, 2)
            .reshape(D, S)
        )
        out[toks] = yb[:, : len(toks)].T + b2[e][None, :]
    return out.reshape(B, Sq, D)


# revision 26
# speedup vs baseline: 1.0633x; 1.0479x over previous
"""MoE kernel for Trainium2 (8 NeuronCores) — 8-way feature-split.

Strategy (feature-parallel over DFF, skew-immune):
  - Host sorts the T=4096 tokens by dispatch_order into per-expert column
    blocks (padded to a multiple of 8).  EVERY core sees all token columns,
    but core c computes only its 4 of the 32 DFF f-chunks (512 of 4096 ff
    dims) for every expert:  h_f = gelu(x @ W1[:, f] + b1[f]) entirely
    on-core, partial y_c = sum_f h_f @ W2[f, :].  The host sums the 8
    partial outputs and adds b2.  This is an exact decomposition; per-core
    compute is sum_e ceil8(count_e) columns (~517-equivalent) instead of
    8*max_e(count_e) (~576), and weight traffic stays 16 MB/core (each
    core reads 1/8th of every expert's W1/W2).
  - Device loop per expert block: phase 1 k-outer (PSUM holds the 4
    f-chunks while the 8 k-tiles stream in -> the PE starts ~1us into the
    kernel, paced by DMA only for block 0), then phase 2 chunk-outer
    (yT partial = W2-slice @ h), partial y DMA'd out as float16.
  - All DMAs are contiguous [128, N] slabs (host packs x block-major
    k-inner, y block-major dm-inner, w1/w2 per-block slabs).

Self-contained: hardcodes all shapes from the problem spec.
"""

import os
import sys
from contextlib import ExitStack

import numpy as np

for _p in ("/opt/trn_rl_repo",):
    if _p not in sys.path:
        sys.path.insert(0, _p)

import concourse.bass as bass  # noqa: E402
import concourse.tile as tile  # noqa: E402
from concourse import mybir  # noqa: E402
from concourse.bass_utils import run_bass_kernel_spmd  # noqa: E402

# ---------------------------------------------------------------------------
# Workaround for this walrus build: a Drain instruction with >1 sem wait
# fails codegen ("Too many sync wait commands").  Replace the Tile
# kernel-tail drain with single-wait SP nops followed by a bare drain.
# ---------------------------------------------------------------------------


def _patched_drain_and_barrier(self, tick_clock, wait_clock):
    from concourse.vector_clock import ScopedClock

    nc = self.nc
    probe = nc.sync.nop(nofuse=True)
    wait_clock.add_sem_waits(probe.ins, ScopedClock({None: tick_clock.global_clock}))
    si = probe.ins.sync_info
    waits = list(si.on_wait) if si and si.on_wait else []
    probe.ins.sync_info = mybir.SyncInfo(on_wait=waits[:1], on_update=[])
    for w in waits[1:]:
        n = nc.sync.nop(nofuse=True)
        n.ins.sync_info = mybir.SyncInfo(on_wait=[w], on_update=[])

    nc.sync.drain()
    nc.all_engine_barrier()
    assert self.sems is not None
    popped = nc._tile_sem_poison_stack.pop()
    assert popped is self._sem_poison
    nc.clear_and_free_semaphores(list(self.sems.allocated().values()))
    nc.all_engine_barrier()


tile.TileContext._drain_and_barrier = _patched_drain_and_barrier


def _split_excess_sync_waits(nc, max_waits=1):
    """This walrus build only encodes one sem wait per instruction.  Hoist
    excess waits onto same-engine nops inserted immediately before."""
    for f in nc.m.functions:
        for bb in f.blocks:
            out = []
            for inst in bb.instructions:
                si = inst.sync_info
                if si and si.on_wait and len(si.on_wait) > max_waits:
                    waits = list(si.on_wait)
                    for i in range(max_waits, len(waits), max_waits):
                        n = mybir.InstNoOp(
                            name=f"{inst.name}-waitsplit-{i}", ins=[], outs=[]
                        )
                        n.engine = inst.engine
                        n.sync_info = mybir.SyncInfo(
                            on_wait=waits[i : i + max_waits], on_update=[]
                        )
                        out.append(n)
                    inst.sync_info = mybir.SyncInfo(
                        on_wait=waits[:max_waits], on_update=list(si.on_update or [])
                    )
                out.append(inst)
            bb.instructions[:] = out


# ---------------------------------------------------------------------------

NUM_EXPERTS = 8
D = 1024
DFF = 4096
N_CORES = 8
KD = D // 128  # 8 contraction chunks for matmul 1
DM = D // 128  # 8 output chunks for matmul 2
FPC = (DFF // 128) // N_CORES  # 4 f-chunks per core
WCOL = KD * FPC * 128  # 4096 packed weight columns per block (w1 and w2)

F32 = mybir.dt.float32
F16 = mybir.dt.float16

LAST_EXEC_NS = None
LAST_RESULT = None

_NC_CACHE = {}


def _chunks(S):
    """Split S columns into <=512-wide chunks (PSUM bank limit), sizes
    multiple of 8, all >=256 when S allows (hides LDWEIGHTS)."""
    n = max(1, -(-S // 512))
    base = -(-(-(-S // n)) // 8) * 8
    out = []
    c0 = 0
    while c0 < S:
        cn = min(base, S - c0)
        out.append((c0, cn))
        c0 += cn
    return out


def _build_nc(sizes):
    nb = len(sizes)
    C = sum(sizes)
    nc = bass.Bass()
    xk = nc.declare_dram_parameter("xk", [128, KD * C], F16, isOutput=False)
    w1 = nc.declare_dram_parameter("w1", [128, nb * WCOL], F16, isOutput=False)
    w2 = nc.declare_dram_parameter("w2", [128, nb * WCOL], F16, isOutput=False)
    b1 = nc.declare_dram_parameter("b1", [128, nb * FPC], F32, isOutput=False)
    yk = nc.declare_dram_parameter("yk", [128, DM * C], F16, isOutput=True)

    gelu = mybir.ActivationFunctionType.Gelu_apprx_tanh
    xoff = [0]
    for S in sizes:
        xoff.append(xoff[-1] + KD * S)
    yoff = [0]
    for S in sizes:
        yoff.append(yoff[-1] + DM * S)

    with ExitStack() as ctx:
        tc = ctx.enter_context(tile.TileContext(nc))
        # Pool depths sized for ~512-col blocks; scale down for extremely
        # skewed dispatch so the pools always fit in SBUF (~190KB/partition).
        big = max(sizes) > 1024
        cpool = ctx.enter_context(tc.tile_pool(name="const", bufs=1))
        wpool = ctx.enter_context(tc.tile_pool(name="w", bufs=1 if big else 3))
        xpool = ctx.enter_context(tc.tile_pool(name="x", bufs=1 if big else 3))
        ypool = ctx.enter_context(tc.tile_pool(name="y", bufs=1 if big else 2))
        hpool = ctx.enter_context(tc.tile_pool(name="h", bufs=FPC if big else 2 * FPC))
        pspool = ctx.enter_context(tc.tile_pool(name="ps", bufs=4, space="PSUM"))

        xts, w1ts, w2ts = {}, {}, {}
        # Startup k-batches: 4 triggers (queue holds ~4 outstanding) sized so
        # early k-tiles land fast while the PE is still cold-clocked.
        KB = [(0, 1), (1, 1), (2, 2), (4, KD - 4)] if KD == 8 else [(0, KD)]

        def prefetch_xw1(e):
            if e in xts:
                return
            S = sizes[e]
            # x: scalar-engine HWDGE queue (shared with activations; the
            # ring depth of 3 keeps the trigger's sem wait long-satisfied).
            xt = xpool.tile([128, KD * S], F16, name="xt", tag="x")
            if e == 0:
                # split per k-batch so the PE can start on k=0 at ~1us
                for k0, kn in KB:
                    nc.scalar.dma_start(
                        xt[:, k0 * S : (k0 + kn) * S], xk[:, k0 * S : (k0 + kn) * S]
                    )
            else:
                nc.scalar.dma_start(xt[:], xk[:, xoff[e] : xoff[e + 1]])
            xts[e] = xt
            w1t = wpool.tile([128, WCOL], F16, name="w1t", tag="w1")
            if e == 0:
                for k0, kn in KB:
                    nc.sync.dma_start(
                        w1t[:, k0 * 512 : (k0 + kn) * 512],
                        w1[:, k0 * 512 : (k0 + kn) * 512],
                    )
            else:
                nc.sync.dma_start(w1t[:], w1[:, e * WCOL : (e + 1) * WCOL])
            w1ts[e] = w1t

        def prefetch_w2(e):
            if e in w2ts:
                return
            w2t = wpool.tile([128, WCOL], F16, name="w2t", tag="w2")
            nc.sync.dma_start(w2t[:], w2[:, e * WCOL : (e + 1) * WCOL])
            w2ts[e] = w2t

        # Prelude trigger order = DMA priority order under the DGE rings'
        # round-robin draining: block 0's stream, block 1's stream, then the
        # w2/x needed later.
        prefetch_xw1(0)
        b1_sb = cpool.tile([128, nb * FPC], F32, name="b1_sb")
        nc.scalar.dma_start(b1_sb[:], b1[:, :])
        if nb > 1:
            prefetch_xw1(1)
        prefetch_w2(0)
        if nb > 1:
            prefetch_w2(1)
        if nb > 2:
            prefetch_xw1(2)

        hs_map = {}

        def do_p1(e):
            S = sizes[e]
            xt, w1t = xts.pop(e), w1ts.pop(e)
            chunks = _chunks(S)

            # ---- phase 1: h_f = gelu(x @ W1[:,f] + b1[f]), k-outer ----
            # Chunks processed in groups of 2 using both PSUM tag rings (8
            # banks): doubles the PE work per arriving k-tile, so block 0
            # stays PE-bound even while its x/w1 stream in.
            hs = [hpool.tile([128, S], F16, name="h", tag="h") for _ in range(FPC)]
            for g0 in range(0, len(chunks), 2):
                grp = chunks[g0 : g0 + 2]
                pss = [
                    [
                        pspool.tile(
                            [128, cn], F32, name="ps1", tag=("p1", "p2")[gi]
                        )
                        for f in range(FPC)
                    ]
                    for gi, (c0, cn) in enumerate(grp)
                ]
                for k in range(KD):
                    for f in range(FPC):
                        for gi, (c0, cn) in enumerate(grp):
                            nc.tensor.matmul(
                                pss[gi][f][:, :],
                                w1t[:, k * 512 + f * 128 : k * 512 + (f + 1) * 128],
                                xt[:, k * S + c0 : k * S + c0 + cn],
                                start=(k == 0),
                                stop=(k == KD - 1),
                            )
                for gi, (c0, cn) in enumerate(grp):
                    for f in range(FPC):
                        nc.scalar.activation(
                            hs[f][:, c0 : c0 + cn],
                            pss[gi][f][:, :],
                            gelu,
                            bias=b1_sb[:, e * FPC + f : e * FPC + f + 1],
                            scale=1.0,
                        )
            hs_map[e] = hs

        def do_p2(e):
            S = sizes[e]
            w2t = w2ts.pop(e)
            hs = hs_map.pop(e)
            chunks = _chunks(S)

            # ---- phase 2: y_partial = sum_f h_f @ W2[f,:], chunk-outer ----
            yt = ypool.tile([128, DM * S], F16, name="yt", tag="y")
            last_ci = len(chunks) - 1
            for ci, (c0, cn) in enumerate(chunks):
                for dm in range(DM):
                    ps2 = pspool.tile([128, cn], F32, name="ps2", tag="p2")
                    for f in range(FPC):
                        nc.tensor.matmul(
                            ps2[:, :],
                            w2t[:, f * 1024 + dm * 128 : f * 1024 + (dm + 1) * 128],
                            hs[f][:, c0 : c0 + cn],
                            start=(f == 0),
                            stop=(f == FPC - 1),
                        )
                    nc.vector.tensor_scalar_add(
                        yt[:, dm * S + c0 : dm * S + c0 + cn], ps2[:, :], 0.0
                    )
                    if ci == last_ci:
                        # y out on the scalar queue: issued after this block's
                        # activations, waits only on copies already in flight.
                        # The final block's last two dm go out singly so the
                        # kernel-tail transfer is as small as possible.
                        tail = e == nb - 1 and dm >= 6
                        if tail:
                            nc.scalar.dma_start(
                                yk[:, yoff[e] + dm * S : yoff[e] + (dm + 1) * S],
                                yt[:, dm * S : (dm + 1) * S],
                            )
                        elif dm % 2 == 1 and not (e == nb - 1 and dm == 7):
                            nc.scalar.dma_start(
                                yk[:, yoff[e] + (dm - 1) * S : yoff[e] + (dm + 1) * S],
                                yt[:, (dm - 1) * S : (dm + 1) * S],
                            )

        # Software pipeline: p2(e) is issued one block after p1(e) --
        # p1(0), p1(1), p2(0), p1(2), p2(1), ... -- so every block's w2 has
        # a full extra phase (~8us) to land before the PE needs it, and the
        # old p1(0) -> p2(0) DMA-wait gap is filled with p1(1)'s matmuls.
        # The h ring (2*FPC bufs) holds exactly the two live blocks this
        # needs; the degenerate huge-block layout (hb == FPC) runs serially.
        if nb > 1 and not big:
            do_p1(0)
            do_p1(1)
            for e in range(nb):
                if e + 2 < nb:
                    prefetch_w2(e + 2)
                do_p2(e)
                if e + 2 < nb:
                    if e + 3 < nb:
                        prefetch_xw1(e + 3)
                    do_p1(e + 2)
        else:
            for e in range(nb):
                if e + 2 < nb:
                    prefetch_xw1(e + 2)
                    prefetch_w2(e + 2)
                do_p1(e)
                do_p2(e)

    _split_excess_sync_waits(nc)
    return nc


def _enable_trace_hooks():
    """Register the NTFF profile hook (missing antenv.axon_hooks shim)."""
    import types

    if "antenv.axon_hooks" not in sys.modules:
        mod = types.ModuleType("antenv.axon_hooks")
        mod._hook = None

        def set_axon_ntff_profile_hook(h):
            mod._hook = h

        def get_axon_ntff_profile_hook():
            return mod._hook

        mod.set_axon_ntff_profile_hook = set_axon_ntff_profile_hook
        mod.get_axon_ntff_profile_hook = get_axon_ntff_profile_hook
        sys.modules["antenv.axon_hooks"] = mod
        import antenv

        antenv.axon_hooks = mod
    import antenv.axon_hooks as ah

    if ah.get_axon_ntff_profile_hook() is None:
        from trn_agent_boot.trn_boot import _ntff_profile_via_ctypes

        ah.set_axon_ntff_profile_hook(
            _ntff_profile_via_ctypes("/opt/axon/libaxon_pjrt.so")
        )
    import concourse.bass_utils as bu

    bu.upload_artifacts = lambda tmpdir: "local://skipped"


def kernel(inputs, w1, b1, w2, b2, dispatch_order):
    global LAST_EXEC_NS, LAST_RESULT

    inputs = np.asarray(inputs, dtype=np.float32)
    w1 = np.asarray(w1, dtype=np.float32)
    b1 = np.asarray(b1, dtype=np.float32)
    w2 = np.asarray(w2, dtype=np.float32)
    b2 = np.asarray(b2, dtype=np.float32)
    disp = np.asarray(dispatch_order).astype(np.int64)

    B, Sq, _ = inputs.shape
    T = B * Sq
    x = inputs.reshape(T, D)

    order = np.argsort(disp, kind="stable")
    counts = np.bincount(disp, minlength=NUM_EXPERTS)
    starts = np.zeros(NUM_EXPERTS + 1, dtype=np.int64)
    np.cumsum(counts, out=starts[1:])

    # blocks: experts with tokens, processed big->small (tail = smallest)
    blocks = sorted(
        (e for e in range(NUM_EXPERTS) if counts[e] > 0),
        key=lambda e: (-counts[e], e),
    )
    sizes = tuple(int(-(-counts[e] // 8) * 8) for e in blocks)
    offs = np.zeros(len(sizes) + 1, dtype=np.int64)
    np.cumsum(sizes, out=offs[1:])
    C = int(offs[-1])

    key = sizes
    if key not in _NC_CACHE:
        _NC_CACHE[key] = _build_nc(sizes)
    nc = _NC_CACHE[key]

    # ---- pack x: per block, [128, KD*S] k-inner slabs, concatenated ----
    xk_arr = np.zeros((128, KD * C), dtype=np.float16)
    tok_lists = []
    for bi, e in enumerate(blocks):
        toks = order[starts[e] : starts[e + 1]]
        tok_lists.append(toks)
        S = sizes[bi]
        xb = np.zeros((128, KD, S), dtype=np.float16)
        # x[toks] is [n, 1024]; feature dim k*128+p -> [k, p, n] -> [p, k, n]
        xb[:, :, : len(toks)] = (
            x[toks].T.reshape(KD, 128, len(toks)).transpose(1, 0, 2)
        )
        xk_arr[:, KD * offs[bi] : KD * offs[bi + 1]] = xb.reshape(128, KD * S)

    # ---- per-core weight packs: core c owns f-chunks [c*FPC, (c+1)*FPC) ----
    nb = len(blocks)
    w1_blocks = w1[blocks]  # [nb, 1024, 4096]
    w2_blocks = w2[blocks]  # [nb, 4096, 1024]
    b1_blocks = b1[blocks]  # [nb, 4096]
    in_maps = []
    for c in range(N_CORES):
        ff = slice(c * FPC * 128, (c + 1) * FPC * 128)
        # w1p[p, e*WCOL + k*512 + fl*128 + j] = w1[e][k*128+p, ff.start+fl*128+j]
        w1p = np.ascontiguousarray(
            w1_blocks[:, :, ff]
            .reshape(nb, KD, 128, FPC, 128)
            .transpose(2, 0, 1, 3, 4)
            .reshape(128, nb * WCOL)
        ).astype(np.float16)
        # w2p[p, e*WCOL + fl*1024 + dm*128 + j] = w2[e][ff.start+fl*128+p, dm*128+j]
        w2p = np.ascontiguousarray(
            w2_blocks[:, ff, :]
            .reshape(nb, FPC, 128, DM, 128)
            .transpose(2, 0, 1, 3, 4)
            .reshape(128, nb * WCOL)
        ).astype(np.float16)
        # b1p[p, e*FPC + fl] = b1[e][ff.start + fl*128 + p]
        b1p = np.ascontiguousarray(
            b1_blocks[:, ff].reshape(nb, FPC, 128).transpose(2, 0, 1).reshape(
                128, nb * FPC
            )
        ).astype(np.float32)
        in_maps.append({"xk": xk_arr, "w1": w1p, "w2": w2p, "b1": b1p})

    trace = os.environ.get("MOE_TRACE") == "1"
    kwargs = {}
    if trace:
        _enable_trace_hooks()
        kwargs["trace"] = True
        tmpdir = os.environ.get("MOE_TRACE_DIR")
        if tmpdir:
            os.makedirs(tmpdir, exist_ok=True)
            kwargs["tmpdir"] = tmpdir

    res = run_bass_kernel_spmd(nc, in_maps, list(range(N_CORES)), **kwargs)
    LAST_RESULT = res
    LAST_EXEC_NS = res.exec_time_ns

    # ---- gather: sum the 8 partial outputs, add b2, unsort ----
    ysum = np.zeros((128, DM * C), dtype=np.float32)
    for c in range(N_CORES):
        ysum += res.results[c]["yk"].astype(np.float32)

    out = np.empty((T, D), dtype=np.float32)
    for bi, e in enumerate(blocks):
        toks = tok_lists[bi]
        S = sizes[bi]
        yb = (
            ysum[:, DM * offs[bi] : DM * offs[bi + 1]]
            .reshape(128, DM, S)
            .transpose(1, 0, 2)
            .reshape(D, S)
        )
        out[toks] = yb[:, : len(toks)].T + b2[e][None, :]
    return out.reshape(B, Sq, D)
